# revision 52
# baseline (speedup 1.0000x reference)
"""Multi-head causal attention on 8 TRN2 NeuronCores.

Sharding: tensor-parallel over heads — 16 heads / 8 cores = 2 heads per core.
Each core computes q/k/v projections for its 2 heads (column-sharded QKV
weights), causal attention for those heads over both batch elements, and the
row-sharded slice of the output projection, producing a full-shape partial
output.  Host sums the 8 partials and adds bo + bv @ Wo.T (the per-head value
bias commutes through the output projection because attention rows sum to 1).

All matmuls run in bf16 (1 cycle/row at ANY output width, vs fp32r's 4x
penalty under 256 columns; ~0.5% end-to-end error, tolerance is 2e-2).
Structure:
  - scores computed transposed [k, q] per 512-wide q block; both heads'
    score/exp streams are interleaved pairwise so the ACT engine is
    saturated with exp work from the start of each window
  - attn @ v runs in the [q, d] orientation: stationary = 128x128 e chunk,
    moving = [v | ones] (129 columns) so the softmax denominator comes out
    as column 128 of the same accumulator — no separate ones matmul
  - normalization is a per-PARTITION scalar multiply on DVE; a 128x128 PE
    transpose (bf16) flips attn output back to [d, q] for the out-projection
  - single merged emission stream: attention(qt) needs only projections of
    token tiles t <= qt, so projection chunks of tile t=qt+1, the previous
    block's out-projection, and (at batch end) the next batch's q/k
    projections are interleaved into the score/exp windows as PE filler
  - out-projection PSUM evacuation rotates DVE/ACT (GPSIMD cannot access
    PSUM on hardware); ACT is only used once a window's exps are all emitted
  - output partials are written bf16 (halves output DMA)
"""

import sys

if "/opt/trn_rl_repo" not in sys.path:
    sys.path.insert(0, "/opt/trn_rl_repo")

import numpy as np
import ml_dtypes

import concourse.bass as bass  # noqa: F401  (engine namespaces live on nc)
import concourse.tile as tile
from concourse import bacc, mybir
from concourse.bass_utils import run_bass_kernel_spmd

F32 = mybir.dt.float32
BF16 = mybir.dt.bfloat16
AF = mybir.ActivationFunctionType
ALU = mybir.AluOpType

B, S, E = 2, 2048, 2048
H, D = 16, 128
NCORES = 8
HPC = H // NCORES          # heads per core = 2
M = HPC * D                # local channels per core = 256
EO = E // 128              # 16 contraction chunks
XT = 512                   # token-tile width for projections
NT = S // XT               # 4 token tiles per batch
QT = 512                   # q-tile width for attention
NQT = S // QT              # 4 q-tiles
KPQ = QT // 128            # k-tiles per q-block = 4
ET = 512                   # e-tile width for out-projection
SCALE = 1.0 / float(np.sqrt(D))
MASK_BIAS = -30.0


def build_nc():
    nc = bacc.Bacc(trn_type="TRN2", target_bir_lowering=False, num_swdge_queues=4)

    xT = nc.declare_dram_parameter("xT", [B, E, S], BF16, isOutput=False)
    wq = nc.declare_dram_parameter("wq", [E, M], BF16, isOutput=False)
    wk = nc.declare_dram_parameter("wk", [E, M], BF16, isOutput=False)
    wv = nc.declare_dram_parameter("wv", [E, M], BF16, isOutput=False)
    wo = nc.declare_dram_parameter("wo", [M, E], BF16, isOutput=False)
    bq = nc.declare_dram_parameter("bq", [128, HPC], F32, isOutput=False)
    bk = nc.declare_dram_parameter("bk", [128, HPC], F32, isOutput=False)
    tb = nc.declare_dram_parameter("tb", [128, 128], F32, isOutput=False)
    ident = nc.declare_dram_parameter("ident", [128, 128], BF16, isOutput=False)
    o = nc.declare_dram_parameter("o", [B, S, E], BF16, isOutput=True)

    with tile.TileContext(nc) as tc:
        _body(tc, nc, xT, wq, wk, wv, wo, bq, bk, tb, ident, o)
    nc.compile()
    return nc


def _body(tc, nc, xT, wq, wk, wv, wo, bq, bk, tb, ident, o):
    from contextlib import ExitStack

    ctx = ExitStack()
    with ctx:
        wpool = ctx.enter_context(tc.tile_pool(name="w", bufs=1))
        xpool = ctx.enter_context(tc.tile_pool(name="x", bufs=3))
        qkv = ctx.enter_context(tc.tile_pool(name="qkv", bufs=1))
        epool = ctx.enter_context(tc.tile_pool(name="e", bufs=4))
        otp = ctx.enter_context(tc.tile_pool(name="ot", bufs=1))
        osp = ctx.enter_context(tc.tile_pool(name="os", bufs=12))
        rp = ctx.enter_context(tc.tile_pool(name="r", bufs=6))
        asb = ctx.enter_context(tc.tile_pool(name="as", bufs=4))
        psA = ctx.enter_context(tc.tile_pool(name="psA", bufs=3, space="PSUM"))
        psC = ctx.enter_context(tc.tile_pool(name="psC", bufs=3, space="PSUM"))
        psB = ctx.enter_context(tc.tile_pool(name="psB", bufs=2, space="PSUM"))

        # ---- weights / constants ----
        wq_sb = wpool.tile([128, EO, M], BF16, tag="wq")
        wk_sb = wpool.tile([128, EO, M], BF16, tag="wk")
        wv_sb = wpool.tile([128, EO, M], BF16, tag="wv")
        wo_sb = wpool.tile([128, HPC, E], BF16, tag="wo")
        warm_rhs = rp.tile([128, 128], BF16, tag="wrm")
        nc.vector.memset(warm_rhs[:], 0.0)
        warm_ps = psC.tile([128, 512], F32, tag="sc")

        def warms(n):
            # PE keep-alive: tiny dep-free matmuls (pstate ramp + DMA-wait fill)
            for _ in range(n):
                nc.tensor.matmul(warm_ps[:1, :128], warm_rhs[:, :1],
                                 warm_rhs[:], start=True, stop=True)

        # DMA order = consumption order of the t=0 projection stream:
        # x0/wq first (q matmuls), wk (k), x1 first half, wv (v), x1 rest, wo
        x_first = xpool.tile([128, EO, XT], BF16, tag="x")
        x_01 = xpool.tile([128, EO, XT], BF16, tag="x", name="x_t")
        _xr0 = xT[0].rearrange("(eo p) s -> p eo s", p=128)
        _wqr = wq.rearrange("(eo p) m -> p eo m", p=128)
        _wkr = wk.rearrange("(eo p) m -> p eo m", p=128)
        nc.gpsimd.dma_start(x_first[:, 0:4], _xr0[:, 0:4, 0:XT])
        nc.gpsimd.dma_start(wq_sb[:, 0:4], _wqr[:, 0:4])
        nc.gpsimd.dma_start(x_first[:, 4:8], _xr0[:, 4:8, 0:XT])
        nc.gpsimd.dma_start(wq_sb[:, 4:8], _wqr[:, 4:8])
        nc.gpsimd.dma_start(wk_sb[:, 0:8], _wkr[:, 0:8])
        nc.gpsimd.dma_start(x_first[:, 8:12], _xr0[:, 8:12, 0:XT])
        nc.gpsimd.dma_start(wq_sb[:, 8:16], _wqr[:, 8:16])
        nc.gpsimd.dma_start(x_first[:, 12:16], _xr0[:, 12:16, 0:XT])
        nc.gpsimd.dma_start(wk_sb[:, 8:16], _wkr[:, 8:16])
        nc.gpsimd.dma_start(wv_sb[:], wv.rearrange("(eo p) m -> p eo m", p=128))
        nc.gpsimd.dma_start(x_01[:], _xr0[:, :, XT:2 * XT])
        nc.gpsimd.dma_start(wo_sb[:], wo.rearrange("(h p) e -> p h e", p=128))
        bq_sb = wpool.tile([128, HPC], F32, tag="bq")
        bk_sb = wpool.tile([128, HPC], F32, tag="bk")
        nc.sync.dma_start(bq_sb[:], bq[:])
        nc.sync.dma_start(bk_sb[:], bk[:])
        tb_sb = wpool.tile([128, 128], F32, tag="tb")
        nc.sync.dma_start(tb_sb[:], tb[:])
        id_sb = wpool.tile([128, 128], BF16, tag="id")
        nc.sync.dma_start(id_sb[:], ident[:])

        def alloc_b():
            qT = qkv.tile([128, HPC, S], BF16, tag="qT", name="qT_sb")
            kT = qkv.tile([128, HPC, S], BF16, tag="kT", name="kT_sb")
            # v with a ones column appended per (k-tile, head): [128,kt,h,130]
            v = qkv.tile([128, S // 128, HPC, 130], BF16, tag="v", name="v_sb")
            aT = otp.tile([128, HPC, S], BF16, tag="aT", name="aT_sb")
            return {"qT": qT, "kT": kT, "v": v, "aT": aT}

        def x_tile(b, t):
            xt = xpool.tile([128, EO, XT], BF16, tag="x", name="x_t")
            nc.gpsimd.dma_start(
                xt[:], xT[b].rearrange("(eo p) s -> p eo s", p=128)[
                    :, :, t * XT:(t + 1) * XT]
            )
            return xt

        # ---- emission pieces; each is (est_pe_ns, closure) ----

        def qk_half(st, x_t, ps, w_sb, h, lo, hi, fin, dst_key=None,
                    bias_sb=None, scl=1.0, t=0):
            def emit():
                for eo in range(lo, hi):
                    nc.tensor.matmul(
                        ps[:, :XT],
                        w_sb[:, eo, h * D:(h + 1) * D],
                        x_t[:, eo, :],
                        start=(eo == 0),
                        stop=(fin and eo == EO - 1),
                    )
                if fin:
                    nc.scalar.activation(
                        st[dst_key][:, h, t * XT:(t + 1) * XT],
                        ps[:, :XT],
                        AF.Identity,
                        bias=bias_sb[:, h:h + 1],
                        scale=scl,
                    )
            return emit

        def v_half(st, x_t, ps, sub, lo, hi, fin, t=0):
            def emit():
                for eo in range(lo, hi):
                    nc.tensor.matmul(
                        ps[:, :M],
                        x_t[:, eo, sub * 128:(sub + 1) * 128],
                        wv_sb[:, eo, :],
                        start=(eo == 0),
                        stop=(fin and eo == EO - 1),
                    )
                if fin:
                    kt = t * (XT // 128) + sub
                    nc.vector.tensor_copy(st["v"][:, kt, :, 0:128], ps[:, :M])
            return emit

        def qk_chunks(st, t, x_t):
            chunks = []
            for h in range(HPC):
                for w_sb, dkey, bias_sb, scl in (
                    (wq_sb, "qT", bq_sb, SCALE),
                    (wk_sb, "kT", bk_sb, 1.0),
                ):
                    ps = psA.tile([128, 512], F32, tag="qkv", name="ps_qk")
                    chunks.append((1700, qk_half(st, x_t, ps, w_sb, h, 0, 8,
                                                 False)))
                    chunks.append((1780, qk_half(st, x_t, ps, w_sb, h, 8, EO,
                                                 True, dkey, bias_sb, scl, t)))
            return chunks

        def v_chunks(st, t, x_t):
            chunks = []
            for sub in range(XT // 128):
                ps = psA.tile([128, 512], F32, tag="qkv", name="ps_v")
                chunks.append((860, v_half(st, x_t, ps, sub, 0, 8, False)))
                chunks.append((940, v_half(st, x_t, ps, sub, 8, EO, True, t)))
            return chunks

        evac_ctr = [0]
        evac_act_ok = [False]

        def outproj_chunks(st, b, qt, use_act=False, qi4s=None):
            # evacuation rotates DVE/Pool (and ACT too in final blocks where
            # no exp work is pending) so PSUM drain keeps up with production
            chunks = []
            for qi4 in (range(KPQ) if qi4s is None else qi4s):
                qi = qt * KPQ + qi4
                for et in range(E // ET):
                    def emit(qi=qi, et=et, b=b, st=st, use_act=use_act):
                        ps = psA.tile([128, 512], F32, tag="qkv", name="ps_op")
                        for h in range(HPC):
                            nc.tensor.matmul(
                                ps[:],
                                st["aT"][:, h, qi * 128:(qi + 1) * 128],
                                wo_sb[:, h, et * ET:(et + 1) * ET],
                                start=(h == 0),
                                stop=(h == HPC - 1),
                            )
                        osb = osp.tile([128, 512], BF16, tag="osb", name="osb")
                        # GPSIMD cannot read PSUM on real HW.  During the
                        # score phase ACT copies would queue behind pending
                        # exps and pin the PSUM slot, so use DVE only; once
                        # the window's exps are all emitted, rotate DVE/ACT.
                        # Final inline blocks (use_act) split each evac into
                        # DVE+ACT halves so the PSUM slot drains 2x faster.
                        n = evac_ctr[0] % 2
                        evac_ctr[0] += 1
                        if use_act and evac_act_ok[0]:
                            nc.vector.tensor_copy(osb[:, :256], ps[:, :256])
                            nc.scalar.copy(osb[:, 256:], ps[:, 256:])
                        elif evac_act_ok[0] and n == 1:
                            nc.scalar.copy(osb[:], ps[:])
                        else:
                            nc.vector.tensor_copy(osb[:], ps[:])
                        nc.sync.dma_start(
                            o[b, qi * 128:(qi + 1) * 128,
                              et * ET:(et + 1) * ET],
                            osb[:],
                        )
                    chunks.append((510, emit))
            return chunks

        def emit_scores(st, h, qt, kt, e_sb):
            jj = kt - qt * KPQ
            lo = max(jj, 0) * 128
            sc = psC.tile([128, 512], F32, tag="sc", name="sc")
            nc.tensor.matmul(
                sc[:, lo:],
                st["kT"][:, h, kt * 128:(kt + 1) * 128],
                st["qT"][:, h, qt * QT:(qt + 1) * QT][:, lo:],
                start=True,
                stop=True,
            )
            if jj >= 0:
                nc.vector.tensor_tensor(
                    sc[:, jj * 128:(jj + 1) * 128],
                    sc[:, jj * 128:(jj + 1) * 128],
                    tb_sb[:],
                    ALU.add,
                )
            nc.scalar.activation(e_sb[:, kt, lo:], sc[:, lo:], AF.Exp)

        def emit_norm(st, h, qt, ql, ut):
            """reciprocal + per-row scale + transpose + evac for one q-chunk."""
            qi = qt * KPQ + ql
            rec = rp.tile([128, 1], F32, tag="rec", name="rec")
            nc.vector.reciprocal(rec[:], ut[:, 128:129])
            a_t = asb.tile([128, 128], BF16, tag="a", name="a_t")
            nc.vector.tensor_scalar(a_t[:], ut[:, 0:128], rec[:], None, ALU.mult)
            trp = psB.tile([128, 128], BF16, tag="ut", name="trp")
            nc.tensor.transpose(trp[:], a_t[:], id_sb[:])
            nc.vector.tensor_copy(
                st["aT"][:, h, qi * 128:(qi + 1) * 128], trp[:]
            )

        def av_pass(st, h, qt, pair, e_sb, fill):
            """attn@v for q-chunks (2*pair, 2*pair+1), then their norms."""
            uts = []
            for ql in (2 * pair, 2 * pair + 1):
                ut = psB.tile([128, 132], F32, tag="ut", name="ut")
                uts.append((ql, ut))
            kt_hi = qt * KPQ + 2 * pair + 1
            for kt in range(kt_hi + 1):
                for ql, ut in uts:
                    if kt > qt * KPQ + ql:
                        continue
                    nc.tensor.matmul(
                        ut[:, :129],
                        e_sb[:, kt, ql * 128:(ql + 1) * 128],
                        st["v"][:, kt, h, 0:129],
                        start=(kt == 0),
                        stop=(kt == qt * KPQ + ql),
                    )
                fill(450)
            for ql, ut in uts:
                emit_norm(st, h, qt, ql, ut)

        def make_filler(chunks):
            # credit-based: pops a chunk only when accumulated budget covers
            # its estimate, so large chunks don't overshoot small stall slots
            queue = list(chunks)
            credit = [0.0]

            def fill(budget_ns):
                credit[0] += budget_ns
                while queue and queue[0][0] <= credit[0]:
                    est, fn = queue.pop(0)
                    fn()
                    credit[0] -= est
            return fill, queue

        # ---------------- the merged stream ----------------

        # t=0 projections, interleaved with PE warm-up while DMAs land
        st0 = alloc_b()
        nc.vector.memset(st0["v"][:, :, :, 128:129], 1.0)
        warms(48)
        t0ps = {}
        for h in range(HPC):
            for key in ("q", "k"):
                t0ps[(key, h)] = psA.tile([128, 512], F32, tag="qkv",
                                          name="ps_qk")
        qk_half(st0, x_first, t0ps[("q", 0)], wq_sb, 0, 0, 4, False)()
        qk_half(st0, x_first, t0ps[("q", 1)], wq_sb, 1, 0, 4, False)()
        warms(6)
        qk_half(st0, x_first, t0ps[("q", 0)], wq_sb, 0, 4, 8, False)()
        qk_half(st0, x_first, t0ps[("q", 1)], wq_sb, 1, 4, 8, False)()
        qk_half(st0, x_first, t0ps[("k", 0)], wk_sb, 0, 0, 8, False)()
        qk_half(st0, x_first, t0ps[("k", 1)], wk_sb, 1, 0, 8, False)()
        warms(14)
        qk_half(st0, x_first, t0ps[("q", 0)], wq_sb, 0, 8, 12, False)()
        qk_half(st0, x_first, t0ps[("q", 1)], wq_sb, 1, 8, 12, False)()
        qk_half(st0, x_first, t0ps[("q", 0)], wq_sb, 0, 12, EO, True, "qT",
                bq_sb, SCALE, 0)()
        qk_half(st0, x_first, t0ps[("q", 1)], wq_sb, 1, 12, EO, True, "qT",
                bq_sb, SCALE, 0)()
        qk_half(st0, x_first, t0ps[("k", 0)], wk_sb, 0, 8, EO, True, "kT",
                bk_sb, 1.0, 0)()
        qk_half(st0, x_first, t0ps[("k", 1)], wk_sb, 1, 8, EO, True, "kT",
                bk_sb, 1.0, 0)()
        # v(t0) must be fully emitted before window 0's attn@v reads it:
        # filler emission order is not a dependency the tile framework sees
        for est, fn in v_chunks(st0, 0, x_first):
            fn()

        xtiles = {(0, 1): x_01}
        # x DMA for the tile consumed by window i+1 is issued at window i
        issue_at = {(0, 0): (0, 2), (0, 1): (0, 3), (0, 2): (1, 0),
                    (0, 3): (1, 1), (1, 0): (1, 2), (1, 1): (1, 3)}

        sts = {0: st0, 1: None}
        for b in range(B):
            st = sts[b]
            for qt in range(NQT):
                nkt = (qt + 1) * KPQ
                if (b, qt) in issue_at:
                    tb_, tt_ = issue_at[(b, qt)]
                    xtiles[(tb_, tt_)] = x_tile(tb_, tt_)
                # fillers for this window (all data-ready by construction)
                chunks = []
                # ALL out-projection fillers go to window 3: it has no
                # projection surplus, and its ACT evacs cannot delay exps
                # of a following window in the same batch
                if qt == 3:
                    chunks += outproj_chunks(st, b, 0)
                    chunks += outproj_chunks(st, b, 1)
                    chunks += outproj_chunks(st, b, 2)
                if qt + 1 < NT:
                    xt = xtiles.pop((b, qt + 1))
                    chunks += v_chunks(st, qt + 1, xt)
                    chunks += qk_chunks(st, qt + 1, xt)
                if b == 0 and qt == NQT - 1:
                    # next batch's q/k projections fill the final exp backlog
                    sts[1] = alloc_b()
                    chunks += qk_chunks(sts[1], 0, xtiles[(1, 0)])
                fill, queue = make_filler(chunks)

                e0 = epool.tile([128, S // 128, 512], BF16, tag="e", name="e0")
                e1 = epool.tile([128, S // 128, 512], BF16, tag="e", name="e1")
                evac_act_ok[0] = False
                for kt in range(nkt):
                    emit_scores(st, 0, qt, kt, e0)
                    fill(420)
                    emit_scores(st, 1, qt, kt, e1)
                    fill(420)
                if qt == NQT - 1:
                    evac_act_ok[0] = True
                av_pass(st, 0, qt, 0, e0, fill)
                fill(900)
                av_pass(st, 0, qt, 1, e0, fill)
                fill(900)
                av_pass(st, 1, qt, 0, e1, fill)
                if qt == NQT - 1:
                    # q-chunk pair 0 of the final block's out-projection is
                    # ready as soon as both heads' pair-0 norms land
                    for est, fn in outproj_chunks(st, b, qt, use_act=True,
                                                  qi4s=(0, 1)):
                        fn()
                fill(900)
                av_pass(st, 1, qt, 1, e1, fill)
                if qt == NQT - 1:
                    for est, fn in outproj_chunks(st, b, qt, use_act=True,
                                                  qi4s=(2, 3)):
                        fn()
                while queue:
                    _, fn = queue.pop(0)
                    fn()
            if b == 0:
                nc.vector.memset(sts[1]["v"][:, :, :, 128:129], 1.0)
                for est, fn in v_chunks(sts[1], 0, xtiles.pop((1, 0))):
                    fn()


_NC_CACHE = None


def _get_nc():
    global _NC_CACHE
    if _NC_CACHE is None:
        _NC_CACHE = build_nc()
    return _NC_CACHE


def _prep_inputs(x, Wq, bq, Wk, bk, Wv, bv, Wo, bo):
    bf16 = ml_dtypes.bfloat16
    x = np.asarray(x, dtype=np.float32)
    xT = np.ascontiguousarray(x.transpose(0, 2, 1).astype(bf16))
    tb_np = np.where(
        np.arange(128)[:, None] <= np.arange(128)[None, :], 0.0, MASK_BIAS
    ).astype(np.float32)
    id_np = np.eye(128, dtype=bf16)
    in_maps = []
    for c in range(NCORES):
        sl = slice(c * M, (c + 1) * M)
        in_maps.append({
            "xT": xT,
            "wq": np.ascontiguousarray(np.asarray(Wq)[sl, :].T.astype(bf16)),
            "wk": np.ascontiguousarray(np.asarray(Wk)[sl, :].T.astype(bf16)),
            "wv": np.ascontiguousarray(np.asarray(Wv)[sl, :].T.astype(bf16)),
            "wo": np.ascontiguousarray(np.asarray(Wo)[:, sl].T.astype(bf16)),
            "bq": np.ascontiguousarray(
                (np.asarray(bq)[sl].astype(np.float32) * SCALE).reshape(HPC, 128).T
            ),
            "bk": np.ascontiguousarray(
                np.asarray(bk)[sl].astype(np.float32).reshape(HPC, 128).T
            ),
            "tb": tb_np,
            "ident": id_np,
        })
    return in_maps


def run(inputs, trace=False):
    in_maps = _prep_inputs(
        inputs["x"], inputs["Wq"], inputs["bq"], inputs["Wk"], inputs["bk"],
        inputs["Wv"], inputs["bv"], inputs["Wo"], inputs["bo"],
    )
    nc = _get_nc()
    res = run_bass_kernel_spmd(nc, in_maps, list(range(NCORES)), trace=trace)
    acc = np.zeros((B, S, E), dtype=np.float64)
    for r in res.results:
        acc += np.asarray(r["o"]).astype(np.float64)
    acc += np.asarray(inputs["bo"], dtype=np.float64)[None, None, :]
    acc += (np.asarray(inputs["bv"], dtype=np.float64)
            @ np.asarray(inputs["Wo"], dtype=np.float64).T)[None, None, :]
    return acc.astype(np.float32), res


def kernel(**inputs):
    out, _ = run(inputs, trace=False)
    return out


# revision 53
# speedup vs baseline: 1.0060x; 1.0060x over previous
"""Multi-head causal attention on 8 TRN2 NeuronCores.

Sharding: tensor-parallel over heads — 16 heads / 8 cores = 2 heads per core.
Each core computes q/k/v projections for its 2 heads (column-sharded QKV
weights), causal attention for those heads over both batch elements, and the
row-sharded slice of the output projection, producing a full-shape partial
output.  Host sums the 8 partials and adds bo + bv @ Wo.T (the per-head value
bias commutes through the output projection because attention rows sum to 1).

All matmuls run in bf16 (1 cycle/row at ANY output width, vs fp32r's 4x
penalty under 256 columns; ~0.5% end-to-end error, tolerance is 2e-2).
Structure:
  - scores computed transposed [k, q] per 512-wide q block; both heads'
    score/exp streams are interleaved pairwise so the ACT engine is
    saturated with exp work from the start of each window
  - attn @ v runs in the [q, d] orientation: stationary = 128x128 e chunk,
    moving = [v | ones] (129 columns) so the softmax denominator comes out
    as column 128 of the same accumulator — no separate ones matmul
  - normalization is a per-PARTITION scalar multiply on DVE; a 128x128 PE
    transpose (bf16) flips attn output back to [d, q] for the out-projection
  - single merged emission stream: attention(qt) needs only projections of
    token tiles t <= qt, so projection chunks of tile t=qt+1, the previous
    block's out-projection, and (at batch end) the next batch's q/k
    projections are interleaved into the score/exp windows as PE filler
  - out-projection PSUM evacuation rotates DVE/ACT (GPSIMD cannot access
    PSUM on hardware); ACT is only used once a window's exps are all emitted
  - output partials are written bf16 (halves output DMA)
"""

import sys

if "/opt/trn_rl_repo" not in sys.path:
    sys.path.insert(0, "/opt/trn_rl_repo")

import numpy as np
import ml_dtypes

import concourse.bass as bass  # noqa: F401  (engine namespaces live on nc)
import concourse.tile as tile
from concourse import bacc, mybir
from concourse.bass_utils import run_bass_kernel_spmd

F32 = mybir.dt.float32
BF16 = mybir.dt.bfloat16
AF = mybir.ActivationFunctionType
ALU = mybir.AluOpType

B, S, E = 2, 2048, 2048
H, D = 16, 128
NCORES = 8
HPC = H // NCORES          # heads per core = 2
M = HPC * D                # local channels per core = 256
EO = E // 128              # 16 contraction chunks
XT = 512                   # token-tile width for projections
NT = S // XT               # 4 token tiles per batch
QT = 512                   # q-tile width for attention
NQT = S // QT              # 4 q-tiles
KPQ = QT // 128            # k-tiles per q-block = 4
ET = 512                   # e-tile width for out-projection
SCALE = 1.0 / float(np.sqrt(D))
MASK_BIAS = -30.0


def build_nc():
    nc = bacc.Bacc(trn_type="TRN2", target_bir_lowering=False, num_swdge_queues=4)

    xT = nc.declare_dram_parameter("xT", [B, E, S], BF16, isOutput=False)
    wq = nc.declare_dram_parameter("wq", [E, M], BF16, isOutput=False)
    wk = nc.declare_dram_parameter("wk", [E, M], BF16, isOutput=False)
    wv = nc.declare_dram_parameter("wv", [E, M], BF16, isOutput=False)
    wo = nc.declare_dram_parameter("wo", [M, E], BF16, isOutput=False)
    bq = nc.declare_dram_parameter("bq", [128, HPC], F32, isOutput=False)
    bk = nc.declare_dram_parameter("bk", [128, HPC], F32, isOutput=False)
    tb = nc.declare_dram_parameter("tb", [128, 128], F32, isOutput=False)
    ident = nc.declare_dram_parameter("ident", [128, 128], BF16, isOutput=False)
    o = nc.declare_dram_parameter("o", [B, S, E], BF16, isOutput=True)

    with tile.TileContext(nc) as tc:
        _body(tc, nc, xT, wq, wk, wv, wo, bq, bk, tb, ident, o)
    nc.compile()
    return nc


def _body(tc, nc, xT, wq, wk, wv, wo, bq, bk, tb, ident, o):
    from contextlib import ExitStack

    ctx = ExitStack()
    with ctx:
        wpool = ctx.enter_context(tc.tile_pool(name="w", bufs=1))
        xpool = ctx.enter_context(tc.tile_pool(name="x", bufs=3))
        qkv = ctx.enter_context(tc.tile_pool(name="qkv", bufs=1))
        epool = ctx.enter_context(tc.tile_pool(name="e", bufs=4))
        otp = ctx.enter_context(tc.tile_pool(name="ot", bufs=1))
        osp = ctx.enter_context(tc.tile_pool(name="os", bufs=12))
        rp = ctx.enter_context(tc.tile_pool(name="r", bufs=6))
        asb = ctx.enter_context(tc.tile_pool(name="as", bufs=4))
        psA = ctx.enter_context(tc.tile_pool(name="psA", bufs=3, space="PSUM"))
        psC = ctx.enter_context(tc.tile_pool(name="psC", bufs=3, space="PSUM"))
        psB = ctx.enter_context(tc.tile_pool(name="psB", bufs=2, space="PSUM"))

        # ---- weights / constants ----
        wq_sb = wpool.tile([128, EO, M], BF16, tag="wq")
        wk_sb = wpool.tile([128, EO, M], BF16, tag="wk")
        wv_sb = wpool.tile([128, EO, M], BF16, tag="wv")
        wo_sb = wpool.tile([128, HPC, E], BF16, tag="wo")
        warm_rhs = rp.tile([128, 128], BF16, tag="wrm")
        nc.vector.memset(warm_rhs[:], 0.0)
        warm_ps = psC.tile([128, 512], F32, tag="sc")

        def warms(n):
            # PE keep-alive: tiny dep-free matmuls (pstate ramp + DMA-wait fill)
            for _ in range(n):
                nc.tensor.matmul(warm_ps[:1, :128], warm_rhs[:, :1],
                                 warm_rhs[:], start=True, stop=True)

        # DMA order = consumption order of the t=0 projection stream:
        # x0/wq first (q matmuls), wk (k), x1 first half, wv (v), x1 rest, wo
        x_first = xpool.tile([128, EO, XT], BF16, tag="x")
        x_01 = xpool.tile([128, EO, XT], BF16, tag="x", name="x_t")
        _xr0 = xT[0].rearrange("(eo p) s -> p eo s", p=128)
        _wqr = wq.rearrange("(eo p) m -> p eo m", p=128)
        _wkr = wk.rearrange("(eo p) m -> p eo m", p=128)
        nc.gpsimd.dma_start(x_first[:, 0:4], _xr0[:, 0:4, 0:XT])
        nc.gpsimd.dma_start(wq_sb[:, 0:4], _wqr[:, 0:4])
        nc.gpsimd.dma_start(x_first[:, 4:8], _xr0[:, 4:8, 0:XT])
        nc.gpsimd.dma_start(wq_sb[:, 4:8], _wqr[:, 4:8])
        nc.gpsimd.dma_start(wk_sb[:, 0:8], _wkr[:, 0:8])
        nc.gpsimd.dma_start(x_first[:, 8:12], _xr0[:, 8:12, 0:XT])
        nc.gpsimd.dma_start(wq_sb[:, 8:16], _wqr[:, 8:16])
        nc.gpsimd.dma_start(x_first[:, 12:16], _xr0[:, 12:16, 0:XT])
        nc.gpsimd.dma_start(wk_sb[:, 8:16], _wkr[:, 8:16])
        nc.gpsimd.dma_start(wv_sb[:], wv.rearrange("(eo p) m -> p eo m", p=128))
        nc.gpsimd.dma_start(x_01[:], _xr0[:, :, XT:2 * XT])
        nc.gpsimd.dma_start(wo_sb[:], wo.rearrange("(h p) e -> p h e", p=128))
        bq_sb = wpool.tile([128, HPC], F32, tag="bq")
        bk_sb = wpool.tile([128, HPC], F32, tag="bk")
        nc.sync.dma_start(bq_sb[:], bq[:])
        nc.sync.dma_start(bk_sb[:], bk[:])
        tb_sb = wpool.tile([128, 128], F32, tag="tb")
        nc.sync.dma_start(tb_sb[:], tb[:])
        id_sb = wpool.tile([128, 128], BF16, tag="id")
        nc.sync.dma_start(id_sb[:], ident[:])

        def alloc_b():
            qT = qkv.tile([128, HPC, S], BF16, tag="qT", name="qT_sb")
            kT = qkv.tile([128, HPC, S], BF16, tag="kT", name="kT_sb")
            # v with a ones column appended per (k-tile, head): [128,kt,h,130]
            v = qkv.tile([128, S // 128, HPC, 130], BF16, tag="v", name="v_sb")
            aT = otp.tile([128, HPC, S], BF16, tag="aT", name="aT_sb")
            return {"qT": qT, "kT": kT, "v": v, "aT": aT}

        def x_tile(b, t):
            xt = xpool.tile([128, EO, XT], BF16, tag="x", name="x_t")
            nc.gpsimd.dma_start(
                xt[:], xT[b].rearrange("(eo p) s -> p eo s", p=128)[
                    :, :, t * XT:(t + 1) * XT]
            )
            return xt

        # ---- emission pieces; each is (est_pe_ns, closure) ----

        def qk_half(st, x_t, ps, w_sb, h, lo, hi, fin, dst_key=None,
                    bias_sb=None, scl=1.0, t=0):
            def emit():
                for eo in range(lo, hi):
                    nc.tensor.matmul(
                        ps[:, :XT],
                        w_sb[:, eo, h * D:(h + 1) * D],
                        x_t[:, eo, :],
                        start=(eo == 0),
                        stop=(fin and eo == EO - 1),
                    )
                if fin:
                    nc.scalar.activation(
                        st[dst_key][:, h, t * XT:(t + 1) * XT],
                        ps[:, :XT],
                        AF.Identity,
                        bias=bias_sb[:, h:h + 1],
                        scale=scl,
                    )
            return emit

        def v_half(st, x_t, ps, sub, lo, hi, fin, t=0):
            def emit():
                for eo in range(lo, hi):
                    nc.tensor.matmul(
                        ps[:, :M],
                        x_t[:, eo, sub * 128:(sub + 1) * 128],
                        wv_sb[:, eo, :],
                        start=(eo == 0),
                        stop=(fin and eo == EO - 1),
                    )
                if fin:
                    kt = t * (XT // 128) + sub
                    nc.vector.tensor_copy(st["v"][:, kt, :, 0:128], ps[:, :M])
            return emit

        def qk_chunks(st, t, x_t):
            chunks = []
            for h in range(HPC):
                for w_sb, dkey, bias_sb, scl in (
                    (wq_sb, "qT", bq_sb, SCALE),
                    (wk_sb, "kT", bk_sb, 1.0),
                ):
                    ps = psA.tile([128, 512], F32, tag="qkv", name="ps_qk")
                    chunks.append((1700, qk_half(st, x_t, ps, w_sb, h, 0, 8,
                                                 False)))
                    chunks.append((1780, qk_half(st, x_t, ps, w_sb, h, 8, EO,
                                                 True, dkey, bias_sb, scl, t)))
            return chunks

        def v_chunks(st, t, x_t):
            chunks = []
            for sub in range(XT // 128):
                ps = psA.tile([128, 512], F32, tag="qkv", name="ps_v")
                chunks.append((860, v_half(st, x_t, ps, sub, 0, 8, False)))
                chunks.append((940, v_half(st, x_t, ps, sub, 8, EO, True, t)))
            return chunks

        evac_ctr = [0]
        evac_act_ok = [False]

        def outproj_chunks(st, b, qt, use_act=False, qi4s=None):
            # evacuation rotates DVE/Pool (and ACT too in final blocks where
            # no exp work is pending) so PSUM drain keeps up with production
            chunks = []
            for qi4 in (range(KPQ) if qi4s is None else qi4s):
                qi = qt * KPQ + qi4
                for et in range(E // ET):
                    def emit(qi=qi, et=et, b=b, st=st, use_act=use_act):
                        ps = psA.tile([128, 512], F32, tag="qkv", name="ps_op")
                        for h in range(HPC):
                            nc.tensor.matmul(
                                ps[:],
                                st["aT"][:, h, qi * 128:(qi + 1) * 128],
                                wo_sb[:, h, et * ET:(et + 1) * ET],
                                start=(h == 0),
                                stop=(h == HPC - 1),
                            )
                        osb = osp.tile([128, 512], BF16, tag="osb", name="osb")
                        # GPSIMD cannot read PSUM on real HW.  During the
                        # score phase ACT copies would queue behind pending
                        # exps and pin the PSUM slot, so use DVE only; once
                        # the window's exps are all emitted, rotate DVE/ACT.
                        n = evac_ctr[0] % 2
                        evac_ctr[0] += 1
                        if evac_act_ok[0] and n == 1:
                            nc.scalar.copy(osb[:], ps[:])
                        else:
                            nc.vector.tensor_copy(osb[:], ps[:])
                        nc.sync.dma_start(
                            o[b, qi * 128:(qi + 1) * 128,
                              et * ET:(et + 1) * ET],
                            osb[:],
                        )
                    chunks.append((510, emit))
            return chunks

        def emit_scores(st, h, qt, kt, e_sb):
            jj = kt - qt * KPQ
            lo = max(jj, 0) * 128
            sc = psC.tile([128, 512], F32, tag="sc", name="sc")
            nc.tensor.matmul(
                sc[:, lo:],
                st["kT"][:, h, kt * 128:(kt + 1) * 128],
                st["qT"][:, h, qt * QT:(qt + 1) * QT][:, lo:],
                start=True,
                stop=True,
            )
            if jj >= 0:
                nc.vector.tensor_tensor(
                    sc[:, jj * 128:(jj + 1) * 128],
                    sc[:, jj * 128:(jj + 1) * 128],
                    tb_sb[:],
                    ALU.add,
                )
            nc.scalar.activation(e_sb[:, kt, lo:], sc[:, lo:], AF.Exp)

        def emit_norm(st, h, qt, ql, ut):
            """reciprocal + per-row scale + transpose + evac for one q-chunk."""
            qi = qt * KPQ + ql
            rec = rp.tile([128, 1], F32, tag="rec", name="rec")
            nc.vector.reciprocal(rec[:], ut[:, 128:129])
            a_t = asb.tile([128, 128], BF16, tag="a", name="a_t")
            nc.vector.tensor_scalar(a_t[:], ut[:, 0:128], rec[:], None, ALU.mult)
            trp = psB.tile([128, 128], BF16, tag="ut", name="trp")
            nc.tensor.transpose(trp[:], a_t[:], id_sb[:])
            nc.vector.tensor_copy(
                st["aT"][:, h, qi * 128:(qi + 1) * 128], trp[:]
            )

        def av_pass(st, h, qt, pair, e_sb, fill):
            """attn@v for q-chunks (2*pair, 2*pair+1), then their norms."""
            uts = []
            for ql in (2 * pair, 2 * pair + 1):
                ut = psB.tile([128, 132], F32, tag="ut", name="ut")
                uts.append((ql, ut))
            kt_hi = qt * KPQ + 2 * pair + 1
            for kt in range(kt_hi + 1):
                for ql, ut in uts:
                    if kt > qt * KPQ + ql:
                        continue
                    nc.tensor.matmul(
                        ut[:, :129],
                        e_sb[:, kt, ql * 128:(ql + 1) * 128],
                        st["v"][:, kt, h, 0:129],
                        start=(kt == 0),
                        stop=(kt == qt * KPQ + ql),
                    )
                fill(450)
            for ql, ut in uts:
                emit_norm(st, h, qt, ql, ut)

        def make_filler(chunks):
            # credit-based: pops a chunk only when accumulated budget covers
            # its estimate, so large chunks don't overshoot small stall slots
            queue = list(chunks)
            credit = [0.0]

            def fill(budget_ns):
                credit[0] += budget_ns
                while queue and queue[0][0] <= credit[0]:
                    est, fn = queue.pop(0)
                    fn()
                    credit[0] -= est
            return fill, queue

        # ---------------- the merged stream ----------------

        # t=0 projections, interleaved with PE warm-up while DMAs land
        st0 = alloc_b()
        nc.vector.memset(st0["v"][:, :, :, 128:129], 1.0)
        warms(48)
        t0ps = {}
        for h in range(HPC):
            for key in ("q", "k"):
                t0ps[(key, h)] = psA.tile([128, 512], F32, tag="qkv",
                                          name="ps_qk")
        qk_half(st0, x_first, t0ps[("q", 0)], wq_sb, 0, 0, 4, False)()
        qk_half(st0, x_first, t0ps[("q", 1)], wq_sb, 1, 0, 4, False)()
        warms(6)
        qk_half(st0, x_first, t0ps[("q", 0)], wq_sb, 0, 4, 8, False)()
        qk_half(st0, x_first, t0ps[("q", 1)], wq_sb, 1, 4, 8, False)()
        qk_half(st0, x_first, t0ps[("k", 0)], wk_sb, 0, 0, 8, False)()
        qk_half(st0, x_first, t0ps[("k", 1)], wk_sb, 1, 0, 8, False)()
        warms(14)
        qk_half(st0, x_first, t0ps[("q", 0)], wq_sb, 0, 8, 12, False)()
        qk_half(st0, x_first, t0ps[("q", 1)], wq_sb, 1, 8, 12, False)()
        qk_half(st0, x_first, t0ps[("q", 0)], wq_sb, 0, 12, EO, True, "qT",
                bq_sb, SCALE, 0)()
        qk_half(st0, x_first, t0ps[("q", 1)], wq_sb, 1, 12, EO, True, "qT",
                bq_sb, SCALE, 0)()
        qk_half(st0, x_first, t0ps[("k", 0)], wk_sb, 0, 8, EO, True, "kT",
                bk_sb, 1.0, 0)()
        qk_half(st0, x_first, t0ps[("k", 1)], wk_sb, 1, 8, EO, True, "kT",
                bk_sb, 1.0, 0)()
        # v(t0) must be fully emitted before window 0's attn@v reads it:
        # filler emission order is not a dependency the tile framework sees
        for est, fn in v_chunks(st0, 0, x_first):
            fn()

        xtiles = {(0, 1): x_01}
        # x DMA for the tile consumed by window i+1 is issued at window i
        issue_at = {(0, 0): (0, 2), (0, 1): (0, 3), (0, 2): (1, 0),
                    (0, 3): (1, 1), (1, 0): (1, 2), (1, 1): (1, 3)}

        sts = {0: st0, 1: None}
        for b in range(B):
            st = sts[b]
            for qt in range(NQT):
                nkt = (qt + 1) * KPQ
                if (b, qt) in issue_at:
                    tb_, tt_ = issue_at[(b, qt)]
                    xtiles[(tb_, tt_)] = x_tile(tb_, tt_)
                # fillers for this window (all data-ready by construction)
                chunks = []
                # ALL out-projection fillers go to window 3: it has no
                # projection surplus, and its ACT evacs cannot delay exps
                # of a following window in the same batch
                if qt == 3:
                    chunks += outproj_chunks(st, b, 0)
                    chunks += outproj_chunks(st, b, 1)
                    chunks += outproj_chunks(st, b, 2)
                if qt + 1 < NT:
                    xt = xtiles.pop((b, qt + 1))
                    chunks += v_chunks(st, qt + 1, xt)
                    chunks += qk_chunks(st, qt + 1, xt)
                if b == 0 and qt == NQT - 1:
                    # next batch's q/k projections fill the final exp backlog
                    sts[1] = alloc_b()
                    chunks += qk_chunks(sts[1], 0, xtiles[(1, 0)])
                fill, queue = make_filler(chunks)

                e0 = epool.tile([128, S // 128, 512], BF16, tag="e", name="e0")
                e1 = epool.tile([128, S // 128, 512], BF16, tag="e", name="e1")
                evac_act_ok[0] = False
                for kt in range(nkt):
                    emit_scores(st, 0, qt, kt, e0)
                    fill(420)
                    emit_scores(st, 1, qt, kt, e1)
                    fill(420)
                if qt == NQT - 1:
                    evac_act_ok[0] = True
                av_pass(st, 0, qt, 0, e0, fill)
                fill(900)
                av_pass(st, 0, qt, 1, e0, fill)
                fill(900)
                av_pass(st, 1, qt, 0, e1, fill)
                if qt == NQT - 1:
                    # q-chunk pair 0 of the final block's out-projection is
                    # ready as soon as both heads' pair-0 norms land
                    for est, fn in outproj_chunks(st, b, qt, use_act=True,
                                                  qi4s=(0, 1)):
                        fn()
                fill(900)
                av_pass(st, 1, qt, 1, e1, fill)
                if qt == NQT - 1:
                    for est, fn in outproj_chunks(st, b, qt, use_act=True,
                                                  qi4s=(2, 3)):
                        fn()
                while queue:
                    _, fn = queue.pop(0)
                    fn()
            if b == 0:
                nc.vector.memset(sts[1]["v"][:, :, :, 128:129], 1.0)
                for est, fn in v_chunks(sts[1], 0, xtiles.pop((1, 0))):
                    fn()


_NC_CACHE = None


def _get_nc():
    global _NC_CACHE
    if _NC_CACHE is None:
        _NC_CACHE = build_nc()
    return _NC_CACHE


def _prep_inputs(x, Wq, bq, Wk, bk, Wv, bv, Wo, bo):
    bf16 = ml_dtypes.bfloat16
    x = np.asarray(x, dtype=np.float32)
    xT = np.ascontiguousarray(x.transpose(0, 2, 1).astype(bf16))
    tb_np = np.where(
        np.arange(128)[:, None] <= np.arange(128)[None, :], 0.0, MASK_BIAS
    ).astype(np.float32)
    id_np = np.eye(128, dtype=bf16)
    in_maps = []
    for c in range(NCORES):
        sl = slice(c * M, (c + 1) * M)
        in_maps.append({
            "xT": xT,
            "wq": np.ascontiguousarray(np.asarray(Wq)[sl, :].T.astype(bf16)),
            "wk": np.ascontiguousarray(np.asarray(Wk)[sl, :].T.astype(bf16)),
            "wv": np.ascontiguousarray(np.asarray(Wv)[sl, :].T.astype(bf16)),
            "wo": np.ascontiguousarray(np.asarray(Wo)[:, sl].T.astype(bf16)),
            "bq": np.ascontiguousarray(
                (np.asarray(bq)[sl].astype(np.float32) * SCALE).reshape(HPC, 128).T
            ),
            "bk": np.ascontiguousarray(
                np.asarray(bk)[sl].astype(np.float32).reshape(HPC, 128).T
            ),
            "tb": tb_np,
            "ident": id_np,
        })
    return in_maps


def run(inputs, trace=False):
    in_maps = _prep_inputs(
        inputs["x"], inputs["Wq"], inputs["bq"], inputs["Wk"], inputs["bk"],
        inputs["Wv"], inputs["bv"], inputs["Wo"], inputs["bo"],
    )
    nc = _get_nc()
    res = run_bass_kernel_spmd(nc, in_maps, list(range(NCORES)), trace=trace)
    acc = np.zeros((B, S, E), dtype=np.float64)
    for r in res.results:
        acc += np.asarray(r["o"]).astype(np.float64)
    acc += np.asarray(inputs["bo"], dtype=np.float64)[None, None, :]
    acc += (np.asarray(inputs["bv"], dtype=np.float64)
            @ np.asarray(inputs["Wo"], dtype=np.float64).T)[None, None, :]
    return acc.astype(np.float32), res


def kernel(**inputs):
    out, _ = run(inputs, trace=False)
    return out
